# revision 1
# baseline (speedup 1.0000x reference)
"""AugmentedLstm Trainium2 kernel — 8 NeuronCores, self-contained.

B=32, T=1024, D=768, H=768.
  proj = inputs @ W_in.T + b_in                    [B,T,6H]
  recurrence over T:  ps = h @ W_s.T + b_s         [B,5H]
    i,f,g,o = sig/sig/tanh/sig(pi+ps); c = i*g + f*c; out0 = o*tanh(c)
    hw = sig(pi4+ps4); out = hw*out0 + (1-hw)*pi5 ; y = out*mask
  (h/c freezing past sequence length never affects the masked y output.)

Distribution: tensor-parallel over the hidden dim (TP-6).
  - cores 0..5 each own one 128-wide H-shard (of each gate block);
    cores 6,7 run the same program on zeroed weights (outputs ignored).
  - Phase 1 (input projection, column-split): each core streams all tokens,
    transposes input tiles on the PE (via identity matmul), and computes its
    pi.T slice -> internal DRAM "pi" [128, t, chunk(7), b]; chunks 0-4 gate
    pre-activations, 5 highway bypass, 6 = sequence mask (broadcast across
    partitions with a rank-1 ones x maskrow matmul).
  - Phase 2 (recurrence): all state transposed [H-shard=128, B=32]. Per step
    30 matmuls (bf16 W stationary, arrived h moving), fp32 gates on DVE/ACT,
    h_next cast to bf16 and pushed to all 8 cores' SBUF with
    remote_dma_broadcast into slot = own partition id; 4-deep recv rotation
    (the h data dependency itself provides cross-core flow control).
"""

import sys

for _p in ("/opt/trn_rl_repo", "/opt/pypackages"):
    if _p not in sys.path:
        sys.path.insert(0, _p)

import numpy as np
import ml_dtypes

import concourse.bass as bass
import concourse.mybir as mybir
from concourse import bacc
from concourse.bass_utils import run_bass_kernel_spmd

F32 = mybir.dt.float32
BF16 = mybir.dt.bfloat16
AF = mybir.ActivationFunctionType

B, D, H = 32, 768, 768
NCORES = 8
TPD = 6      # active tensor-parallel cores
HC = 128     # H-shard width per core
NG = 5       # recurrent gate blocks (i,f,g,o,hw)
NPI = 6      # pi blocks per step (5 gates + highway)
NKD = 6      # 128-wide contraction chunks over D=H=768


def build_program(T):
    assert T % 16 == 0
    NTB = T * B // 512          # 512-token blocks in phase 1
    NJ = T // 4                 # phase-2 loop iterations (4 steps each)

    nc = bacc.Bacc("TRN2", target_bir_lowering=False, debug=False,
                   num_devices=NCORES)

    # ---------------- DRAM ----------------
    xin = nc.dram_tensor("xin", [B, T, D], BF16, kind="ExternalInput").ap()
    w1t = nc.dram_tensor("w1t", [D, NPI * HC], BF16, kind="ExternalInput").ap()
    w2t = nc.dram_tensor("w2t", [H, NG * HC], BF16, kind="ExternalInput").ap()
    b1d = nc.dram_tensor("b1", [HC, NPI], F32, kind="ExternalInput").ap()
    b2d = nc.dram_tensor("b2", [HC, NG], F32, kind="ExternalInput").ap()
    identd = nc.dram_tensor("ident", [128, 128], BF16, kind="ExternalInput").ap()
    onesd = nc.dram_tensor("ones1", [1, 128], BF16, kind="ExternalInput").ap()
    mrowd = nc.dram_tensor("mrow", [1, T * 32], BF16, kind="ExternalInput").ap()
    pi = nc.dram_tensor("pi", [128, T + 8, 7, 32], F32, kind="Internal").ap()
    yout = nc.dram_tensor("y", [128, T, 32], F32, kind="ExternalOutput").ap()

    # ---------------- SBUF ----------------
    sb = nc.alloc_sbuf_tensor
    w1_sb = sb("w1_sb", [128, NKD * NPI * HC], BF16)
    w2_sb = sb("w2_sb", [128, NKD * NG * HC], BF16)
    b1_sb = sb("b1_sb", [128, NPI], F32)
    b2_sb = sb("b2_sb", [128, NG], F32)
    id_sb = sb("id_sb", [128, 128], BF16)
    on_sb = sb("on_sb", [1, 128], BF16)
    mr_sb = sb("mr_sb", [1, T * 32], BF16)
    in_sb = [sb(f"in_sb{u}", [128, D], BF16) for u in range(8)]
    rhs_sb = [sb(f"rhs_sb{c}", [128, 2 * 512], BF16) for c in range(NKD)]
    piout = [sb(f"piout{m}", [128, 512], F32) for m in range(2)]
    mout = [sb(f"mout{m}", [128, 512], F32) for m in range(2)]

    recv = [sb(f"recv{s}", [128, NCORES * 32], BF16) for s in range(4)]
    pib = [sb(f"pib{s}", [128, 7 * 32], F32) for s in range(4)]
    send = [sb(f"send{p}", [128, 32], BF16) for p in range(2)]
    ybuf = [sb(f"ybuf{s}", [128, 32], F32) for s in range(4)]
    ctile = sb("ctile", [128, 32], F32)
    sg = [sb(f"sg{i}", [128, 32], F32) for i in range(NG)]
    ag = [sb(f"ag{i}", [128, 32], F32) for i in range(NG)]
    tmp0 = sb("tmp0", [128, 32], F32)
    tmp1 = sb("tmp1", [128, 32], F32)
    tanhc = sb("tanhc", [128, 32], F32)
    out0 = sb("out0", [128, 32], F32)
    htile = sb("htile", [128, 32], F32)

    # ---------------- PSUM ----------------
    ptr = [nc.alloc_psum_tensor(f"ptr{p}", [128, 512], BF16) for p in range(2)]
    pmm = [nc.alloc_psum_tensor(f"pmm{p}", [128, 512], F32) for p in range(2)]
    pmsk = nc.alloc_psum_tensor("pmsk", [128, 512], F32)
    p2 = [nc.alloc_psum_tensor(f"p2_{p}", [128, NG * 32], F32) for p in range(2)]

    # ---------------- semaphores ----------------
    sem = nc.alloc_semaphore
    WLD, TRC, MMD, PIA = sem("WLD"), sem("TRC"), sem("MMD"), sem("PIA")
    INS = [sem("INS0"), sem("INS1")]
    PIS = [sem("PIS0"), sem("PIS1")]
    MSS = [sem("MSS0"), sem("MSS1")]
    PTD, MSD, MSC = sem("PTD"), sem("MSD"), sem("MSC")
    RS = [sem(f"RS{s}") for s in range(4)]
    PID = [sem(f"PID{s}") for s in range(4)]
    YS = [sem(f"YS{s}") for s in range(4)]
    LS = [sem("LS0"), sem("LS1")]
    PR, PSD = sem("PR"), sem("PSD")
    Asem, Bsem, Cd, Dd, Z = (sem("A"), sem("B"), sem("Cd"), sem("Dd"),
                              sem("Z"))
    PF, YB, SD = sem("PF"), sem("YB"), sem("SD")

    tens, vec, scl, gp, syn = nc.tensor, nc.vector, nc.scalar, nc.gpsimd, nc.sync

    def w1tile(kd, m):
        return w1_sb.ap()[:, kd * (NPI * HC) + m * HC:
                          kd * (NPI * HC) + (m + 1) * HC]

    def w2tile(kd, m):
        return w2_sb.ap()[:, kd * (NG * HC) + m * HC:
                          kd * (NG * HC) + (m + 1) * HC]

    # ============ preamble: constant loads ============
    syn.dma_start(w1_sb.ap().rearrange("p (k c) -> p k c", k=NKD),
                  w1t.rearrange("(k p) c -> p k c", p=128)).then_inc(WLD, 16)
    syn.dma_start(w2_sb.ap().rearrange("p (k c) -> p k c", k=NKD),
                  w2t.rearrange("(k p) c -> p k c", p=128)).then_inc(WLD, 16)
    syn.dma_start(b1_sb.ap(), b1d).then_inc(WLD, 16)
    syn.dma_start(b2_sb.ap(), b2d).then_inc(WLD, 16)
    syn.dma_start(id_sb.ap(), identd).then_inc(WLD, 16)
    syn.dma_start(on_sb.ap(), onesd).then_inc(WLD, 16)
    syn.dma_start(mr_sb.ap(), mrowd).then_inc(WLD, 16)
    tens.wait_ge(WLD, 112)
    vec.wait_ge(WLD, 112)
    scl.wait_ge(WLD, 112)

    # ============ phase 1: input projection (python-unrolled) ============
    for tb in range(NTB):
        half = tb % 2
        # token loads: 4 tiles x [128 = 4t x 32b, 768]
        if tb >= 2:
            syn.wait_ge(PTD, 6 * (tb - 1))
        for u in range(4):
            for v in range(4):
                tq = tb * 16 + 4 * u + v
                syn.dma_start(
                    in_sb[4 * half + u].ap()[32 * v:32 * (v + 1), :],
                    xin[:, tq:tq + 1, :],
                ).then_inc(INS[half], 16)
        # PE transposes: 6 chunk-groups of 4
        for c in range(NKD):
            g = 6 * tb + c
            if c == 0:
                tens.wait_ge(INS[half], 256 * (tb // 2 + 1))
            if g >= 2:
                tens.wait_ge(TRC, g - 1)
            for u in range(4):
                mm = tens.transpose(
                    ptr[c % 2].ap()[:, 128 * u:128 * (u + 1)],
                    in_sb[4 * half + u].ap()[:, 128 * c:128 * (c + 1)],
                    id_sb.ap(),
                )
                if u == 3:
                    mm.then_inc(PTD, 1)
        # DVE: psum -> bf16 rhs tiles
        for c in range(NKD):
            g = 6 * tb + c
            vec.wait_ge(PTD, g + 1)
            if tb >= 2 and c == 0:
                vec.wait_ge(MMD, 6 * (tb - 1))
            vec.tensor_copy(
                rhs_sb[c].ap()[:, half * 512:(half + 1) * 512],
                ptr[c % 2].ap(),
            ).then_inc(TRC, 1)
        # PE: 6 m-groups x 6 kd matmuls
        for m in range(NPI):
            g2 = 6 * tb + m
            if m == 0:
                tens.wait_ge(TRC, 6 * (tb + 1))
            if g2 >= 2:
                tens.wait_ge(PIA, g2 - 1)
            for kd in range(NKD):
                mm = tens.matmul(
                    pmm[m % 2].ap(),
                    w1tile(kd, m),
                    rhs_sb[kd].ap()[:, half * 512:(half + 1) * 512],
                    start=(kd == 0),
                    stop=(kd == NKD - 1),
                )
                if kd == NKD - 1:
                    mm.then_inc(MMD, 1)
        # DVE: + b_in, fp32 out; sync: store to pi
        for m in range(NPI):
            g2 = 6 * tb + m
            vec.wait_ge(MMD, g2 + 1)
            if g2 >= 2:
                vec.wait_ge(PIS[g2 % 2], 16 * (g2 // 2))
            vec.tensor_scalar_add(
                piout[m % 2].ap(), pmm[m % 2].ap(), b1_sb.ap()[:, m:m + 1]
            ).then_inc(PIA, 1)
            syn.wait_ge(PIA, g2 + 1)
            syn.dma_start(
                pi[:, tb * 16:(tb + 1) * 16, m:m + 1, :], piout[m % 2].ap()
            ).then_inc(PIS[g2 % 2], 16)
        # mask broadcast for this block: ones[1,128] x mrow[1,512]
        tens.wait_ge(MSC, tb)
        tens.matmul(
            pmsk.ap(), on_sb.ap(),
            mr_sb.ap()[0:1, tb * 512:(tb + 1) * 512],
            start=True, stop=True,
        ).then_inc(MSD, 1)
        vec.wait_ge(MSD, tb + 1)
        if tb >= 2:
            vec.wait_ge(MSS[half], 16 * (tb // 2))
        vec.tensor_copy(mout[half].ap(), pmsk.ap()).then_inc(MSC, 1)
        syn.wait_ge(MSC, tb + 1)
        syn.dma_start(
            pi[:, tb * 16:(tb + 1) * 16, 6:7, :], mout[half].ap()
        ).then_inc(MSS[half], 16)

    for p_ in range(2):
        syn.wait_ge(PIS[p_], 16 * (NPI * NTB // 2))
        syn.wait_ge(MSS[p_], 16 * (NTB // 2))
    # zero-fill the 8 tail rows of pi (read by harmless tail prefetches)
    TZ = sem("TZ")
    for p_ in range(2):
        vec.wait_ge(PIS[p_], 16 * (NPI * NTB // 2))
    vec.drain()
    vec.memset(piout[0].ap()[:, 0:224], 0.0).then_inc(TZ, 1)
    syn.wait_ge(TZ, 1)
    for r_ in range(8):
        syn.dma_start(pi[:, T + r_:T + r_ + 1, :, :],
                      piout[0].ap()[:, 0:224]).then_inc(TZ, 16)
    syn.wait_ge(TZ, 129)
    nc.all_engine_barrier()

    # ============ phase 2: recurrence ============
    pid_sv = gp.partition_id()
    rdests = [(0, k) for k in range(NCORES)]

    # preamble: zero h broadcast into recv[0], zero c, prefetch pi 0..3
    vec.memset(send[1].ap(), 0.0).then_inc(Z, 1)
    vec.memset(ctile.ap(), 0.0)
    vec.sem_inc(PF, 2)
    gp.wait_ge(Z, 1)
    gp.remote_dma_broadcast(
        recv[0].ap()[:, bass.ts(pid_sv, 32)], send[1].ap(),
        remote_sem=RS[0], local_sem=LS[1], rdests=rdests,
    ).then_inc(PR, 1)
    gp.wait_ge(PR, 1)
    gp.trigger_dma(1)
    for s in range(4):
        syn.dma_start(pib[s].ap(), pi[:, s:s + 1, :, :]).then_inc(PID[s], 16)

    with nc.Fori(0, NJ) as j:
        for s in range(4):
            par = s % 2
            # ---- PE: 5 m-tiles x 6 chunks ----
            tens.wait_ge(PF, j * 4 + (s + 1))
            tens.wait_ge(RS[s], j * 16 + 16)
            for m in range(NG):
                for kd in range(NKD):
                    mm = tens.matmul(
                        p2[par].ap()[:, 32 * m:32 * (m + 1)],
                        w2tile(kd, m),
                        recv[s].ap()[:, 32 * kd:32 * (kd + 1)],
                        start=(kd == 0),
                        stop=(kd == NKD - 1),
                    )
                    if kd == NKD - 1:
                        mm.then_inc(PSD, 1)
            # ---- DVE: gate pre-activations ----
            vec.wait_ge(PSD, j * 20 + (5 * s + 5))
            vec.wait_ge(PID[s], j * 16 + 16)
            if True:
                vec.wait_ge(YS[s], j * 16)
                vec.wait_ge(LS[par], j * 32 + (8 * s + (8 if par else 0)))
            for i in range(NG):
                vec.tensor_add(
                    sg[i].ap(), p2[par].ap()[:, 32 * i:32 * (i + 1)],
                    pib[s].ap()[:, 32 * i:32 * (i + 1)],
                ).then_inc(Asem, 1)
            vec.drain().then_inc(PF, 1)
            # ---- ACT: activations with b_s bias ----
            for i in range(NG):
                scl.wait_ge(Asem, j * 20 + (5 * s + i + 1))
                scl.activation(
                    ag[i].ap(), sg[i].ap(),
                    AF.Tanh if i == 2 else AF.Sigmoid,
                    bias=b2_sb.ap()[:, i:i + 1],
                ).then_inc(Bsem, 1)
            # ---- DVE: c update ----
            vec.wait_ge(Bsem, j * 20 + (5 * s + 3))
            vec.tensor_mul(tmp0.ap(), ag[0].ap(), ag[2].ap())
            vec.tensor_mul(tmp1.ap(), ag[1].ap(), ctile.ap())
            vec.drain()
            vec.tensor_add(ctile.ap(), tmp0.ap(), tmp1.ap()).then_inc(Cd, 1)
            scl.wait_ge(Cd, j * 4 + (s + 1))
            scl.activation(tanhc.ap(), ctile.ap(), AF.Tanh).then_inc(Dd, 1)
            # ---- DVE: output, highway, mask, cast ----
            vec.wait_ge(Bsem, j * 20 + (5 * s + 5))
            vec.wait_ge(Dd, j * 4 + (s + 1))
            vec.tensor_mul(out0.ap(), ag[3].ap(), tanhc.ap())
            vec.drain()
            vec.tensor_sub(tmp0.ap(), out0.ap(), pib[s].ap()[:, 160:192])
            vec.drain()
            vec.tensor_mul(tmp1.ap(), ag[4].ap(), tmp0.ap())
            vec.drain()
            vec.tensor_add(htile.ap(), tmp1.ap(), pib[s].ap()[:, 160:192])
            vec.drain()
            vec.tensor_mul(ybuf[s].ap(), htile.ap(),
                           pib[s].ap()[:, 192:224]).then_inc(YB, 1)
            vec.tensor_copy(send[par].ap(), htile.ap()).then_inc(SD, 1)
            # ---- gpsimd: broadcast h_{t+1} ----
            gp.wait_ge(SD, j * 4 + (s + 1))
            gp.remote_dma_broadcast(
                recv[(s + 1) % 4].ap()[:, bass.ts(pid_sv, 32)],
                send[par].ap(),
                remote_sem=RS[(s + 1) % 4], local_sem=LS[par],
                rdests=rdests,
            ).then_inc(PR, 1)
            gp.wait_ge(PR, j * 4 + (s + 2))
            gp.trigger_dma(1)
            # ---- sync: store y, prefetch pi t+4 ----
            syn.wait_ge(YB, j * 4 + (s + 1))
            syn.dma_start(
                yout[:, bass.DynSlice(j * 4 + s, 1), :], ybuf[s].ap()
            ).then_inc(YS[s], 16)
            syn.dma_start(
                pib[s].ap(), pi[:, bass.DynSlice(j * 4 + (s + 4), 1), :, :]
            ).then_inc(PID[s], 16)

    nc.all_engine_barrier()
    nc.compile()
    return nc


# ---------------------------------------------------------------------------
_CACHE = {}


def _get_program(T):
    if T not in _CACHE:
        _CACHE[T] = build_program(T)
    return _CACHE[T]


def make_in_maps(inputs, W_in, b_in, W_s, b_s, lengths, T):
    bf = ml_dtypes.bfloat16
    W_in6 = np.asarray(W_in, np.float32).reshape(NPI, H, D)
    W_s5 = np.asarray(W_s, np.float32).reshape(NG, H, H)
    b_in6 = np.asarray(b_in, np.float32).reshape(NPI, H)
    b_s5 = np.asarray(b_s, np.float32).reshape(NG, H)
    lengths = np.asarray(lengths).astype(np.int64)
    x = np.ascontiguousarray(np.asarray(inputs, np.float32)).astype(bf)

    tt = np.arange(T)[:, None]                       # [T,1]
    mask = (tt < lengths[None, :]).astype(np.float32)  # [T,B]
    mrow = np.ascontiguousarray(mask.reshape(1, T * 32)).astype(bf)
    identm = np.eye(128).astype(bf)
    ones1 = np.ones((1, 128), bf)

    in_maps = []
    for k in range(NCORES):
        if k < TPD:
            w1k = W_in6[:, HC * k:HC * (k + 1), :]       # [6,128,D]
            w1t = np.ascontiguousarray(
                w1k.transpose(2, 0, 1).reshape(D, NPI * HC)).astype(bf)
            w2k = W_s5[:, HC * k:HC * (k + 1), :]        # [5,128,H]
            w2t = np.ascontiguousarray(
                w2k.transpose(2, 0, 1).reshape(H, NG * HC)).astype(bf)
            b1k = np.ascontiguousarray(
                b_in6[:, HC * k:HC * (k + 1)].T).astype(np.float32)
            b2k = np.ascontiguousarray(
                b_s5[:, HC * k:HC * (k + 1)].T).astype(np.float32)
        else:
            w1t = np.zeros((D, NPI * HC), bf)
            w2t = np.zeros((H, NG * HC), bf)
            b1k = np.zeros((HC, NPI), np.float32)
            b2k = np.zeros((HC, NG), np.float32)
        in_maps.append({
            "xin": x, "w1t": w1t, "w2t": w2t, "b1": b1k, "b2": b2k,
            "ident": identm, "ones1": ones1, "mrow": mrow,
        })
    return in_maps


def gather_output(results, T):
    ys = np.stack([results[k]["y"] for k in range(TPD)], axis=0)  # [6,128,T,32]
    yt = ys.reshape(H, T, B)
    return np.ascontiguousarray(yt.transpose(2, 1, 0)).astype(np.float32)


def kernel(inputs, W_in, b_in, W_s, b_s, lengths):
    T = np.asarray(inputs).shape[1]
    nc = _get_program(T)
    in_maps = make_in_maps(inputs, W_in, b_in, W_s, b_s, lengths, T)
    res = run_bass_kernel_spmd(nc, in_maps, core_ids=list(range(NCORES)),
                               trace=False)
    return gather_output(res.results, T)


def kernel_timed(inputs, W_in, b_in, W_s, b_s, lengths, iters=3):
    """Like kernel(), but keeps inputs device-resident and times repeated
    executions (min wall across iters). Returns (output, best_ns, times)."""
    import time
    import jax
    from jax.sharding import Mesh, PartitionSpec
    from jax.experimental.shard_map import shard_map
    from concourse import bass2jax, mybir as _mb

    T = np.asarray(inputs).shape[1]
    nc = _get_program(T)
    in_maps = make_in_maps(inputs, W_in, b_in, W_s, b_s, lengths, T)
    bass2jax.install_neuronx_cc_hook()

    partition_name = (nc.partition_id_tensor.name
                      if nc.partition_id_tensor else None)
    in_names, out_names, out_avals, zero_outs = [], [], [], []
    for alloc in nc.m.functions[0].allocations:
        if not isinstance(alloc, _mb.MemoryLocationSet):
            continue
        name = alloc.memorylocations[0].name
        if alloc.kind == "ExternalInput":
            if name != partition_name:
                in_names.append(name)
        elif alloc.kind == "ExternalOutput":
            shape = tuple(alloc.tensor_shape)
            dtype = _mb.dt.np(alloc.dtype)
            out_names.append(name)
            out_avals.append(jax.core.ShapedArray(shape, dtype))
            zero_outs.append(np.zeros(shape, dtype))
    n_params = len(in_names)
    all_in_names = list(in_names) + list(out_names)
    if partition_name is not None:
        all_in_names.append(partition_name)

    def _body(*args):
        operands = list(args)
        if partition_name is not None:
            operands.append(bass2jax.partition_id_tensor())
        outs = bass2jax._bass_exec_p.bind(
            *operands,
            out_avals=tuple(out_avals),
            in_names=tuple(all_in_names),
            out_names=tuple(out_names),
            lowering_input_output_aliases=(),
            sim_require_finite=True,
            sim_require_nnan=True,
            nc=nc,
        )
        return tuple(outs)

    devices = jax.devices()[:NCORES]
    mesh = Mesh(np.asarray(devices), ("core",))
    n_outs = len(out_names)
    in_specs = (PartitionSpec("core"),) * (n_params + n_outs)
    out_specs = (PartitionSpec("core"),) * n_outs
    donate = tuple(range(n_params, n_params + n_outs))
    sharded = jax.jit(shard_map(_body, mesh=mesh, in_specs=in_specs,
                                out_specs=out_specs, check_rep=False),
                      donate_argnums=donate, keep_unused=True)

    from jax.sharding import NamedSharding
    shard0 = NamedSharding(mesh, PartitionSpec("core"))
    concat_in = [
        jax.device_put(
            np.concatenate([np.asarray(in_maps[c][nm]) for c in range(NCORES)],
                           axis=0), shard0)
        for nm in in_names
    ]
    zero_sets = [
        [jax.device_put(np.zeros((NCORES * z.shape[0], *z.shape[1:]), z.dtype),
                        shard0) for z in zero_outs]
        for _ in range(iters)
    ]
    for a in concat_in + [z for zs in zero_sets for z in zs]:
        a.block_until_ready()

    times = []
    out_arrs = None
    for it in range(iters):
        t0 = time.time()
        out_arrs = sharded(*concat_in, *zero_sets[it])
        for o in out_arrs:
            o.block_until_ready()
        times.append(time.time() - t0)
    results = [
        {name: np.asarray(out_arrs[i]).reshape(NCORES, *out_avals[i].shape)[c]
         for i, name in enumerate(out_names)}
        for c in range(NCORES)
    ]
    return gather_output(results, T), min(times) * 1e9, times


if __name__ == "__main__":
    print("kernel module; call kernel(**inputs)")

